# revision 25
# baseline (speedup 1.0000x reference)
"""Causal multi-head attention (QKV projection + softmax(QK^T)V) on 8 TRN2 NeuronCores.

Problem: x[4,2048,1024] @ W_qkv[1024,3072] + b_qkv -> 16-head causal attention -> [4,2048,1024].

Sharding: core i = (batch bi=i//2, head-group hg=i%2). Each core handles 1 batch x 8 heads,
fully data/tensor-parallel (no collectives). Host pre-arranges per-core inputs (all matmul
operands fp16; accumulation f32 in PSUM):
  - x passed pre-transposed [1024, 2048] so the contraction dim lands on partitions with
    plain contiguous DMAs (no on-device transposes anywhere).
  - wqk [1024,1024] pair-major (pair p: Q cols at 256p, K at 256p+128), head-PAIR-stacked
    (64+64 rows) so QKV^T matmul output chunks are directly the [hd, n] stacked layout the
    attention stage consumes.
  - wv [1024,520]: V columns with per-head stride 65; col 65h+64 is a zero column, and
    the replicated bias tile bv has 1.0 there, so the "ones column" that makes the PV
    matmul accumulate softmax denominators (and b_v itself) ride the DVE PSUM->SBUF
    drain as a tensor_add -- no bias matmuls at all.
Device pipeline per core:
  QKV^T matmuls (Q^T pair-stacked, K^T zero-padded per head so S^T runs K=128 with fast
  weight load -- a 64-row contraction measures ~1.8x slower per matmul, FWL needs the
  full 128 rows) -> S^T = K Q^T per key-chunk with causal column trimming -> one ScalarE
  Exp(scale=1/8) per 2-chunk group (the fully-diagonal group merged into a single act
  over both chunks' 256:512 suffix; the stale sliver is never consumed), PSUM->SBUF
  fp16 = P^T -> causal tri-mask multiply on the 128x128 diagonal blocks only (DVE) ->
  PV matmuls accumulate [q, 64 cols + denominator] per q-block (both heads packed in one
  PSUM bank) -> reciprocal (DVE) * scale (DVE late / ScalarE early) epilogue ->
  DMA out [2048, 512] f32.
Scheduling: ScalarE exp (~163us) and TensorE (~191us) must overlap near-perfectly.
 - A dozen warm-up matmuls on memset scratch run first so the PE HAM clock-gate reaches
   K=8/8 (2.4 GHz) before real work; without them the DMA-gated start keeps the PE at
   1.2 GHz for the first ~20us of real matmuls (HAM needs ~3.4us of sustained activity).
 - Input DMA is batched on the sync ring in first-use-deadline order (the start is
   DMA-bandwidth-bound with all 8 cores pulling at once; big triggers amortize the
   ~620ns trigger serialization): bqk, then x-stripe-0/wqk-pair-0 as two 4-chunk
   batches each (the prologue QK chains are emitted in matching k 0-3 / 4-7 halves),
   then the bulk.
 - kt's zero half-slabs are cleared by 8 per-slab memsets split across DVE (slabs 0-1)
   and gpsimd (slabs 2-7) in slab order, so the first S^T unblocks ~10.5us in instead
   of being gated by a monolithic 13.7us gpsimd memset.
 - Attention runs stripe-major across head-pairs (t-major rounds); QKV matmul tiles are
   a deadline-ordered "filler" queue drained between S^T groups (next block's QK tiles
   are prefetched from the current block's hooks); each stripe's PV matmuls are
   deferred into the next block's S^T/exp loop. PV pops stay whole and early in the
   block so the previous pt tile is consumed before the next block's first exp rotates
   the pt pool. The final block inlines mask+PV per diagonal chunk and pushes its
   output DMAs onto the idle scalar HWDGE ring so the tail doesn't serialize on the
   sync ring's DMA-sem slots.
"""

import numpy as np

import concourse.bass as bass
import concourse.tile as tile
from concourse import bacc, mybir
from concourse import bass_utils

F16 = mybir.dt.float16
F32 = mybir.dt.float32

B, N, D = 4, 2048, 1024
H = 16  # global heads
HD = 64
HL = 8  # heads per core
N_CORES = 8
P = 128
NT = N // P  # 16 token tiles
KC = D // P  # 8 contraction chunks
VW = HL * (HD + 1)  # 520
VH = VW // 2  # 260

_cache = {}


def _build():
    nc = bacc.Bacc("TRN2", target_bir_lowering=False, debug=False)

    x_d = nc.dram_tensor("x", [D, N], F16, kind="ExternalInput").ap()  # x^T, host-transposed
    wqk_d = nc.dram_tensor("wqk", [D, 1024], F16, kind="ExternalInput").ap()
    wv_d = nc.dram_tensor("wv", [D, VW], F16, kind="ExternalInput").ap()
    bqk_d = nc.dram_tensor("bqk", [P, 8], F32, kind="ExternalInput").ap()
    bv_d = nc.dram_tensor("bv", [P, VW], F16, kind="ExternalInput").ap()
    tri_d = nc.dram_tensor("tri", [P, P], F16, kind="ExternalInput").ap()
    out_d = nc.dram_tensor("out", [N, HL * HD], F32, kind="ExternalOutput").ap()

    wqk_r = wqk_d.rearrange("(k p) n -> p k n", p=P)
    wv_r = wv_d.rearrange("(k p) n -> p k n", p=P)

    with tile.TileContext(nc) as tc:
        with (
            tc.tile_pool(name="const", bufs=1) as cpool,
            tc.tile_pool(name="pt", bufs=2) as ptpool,
            tc.tile_pool(name="opair", bufs=6) as oppool,
            tc.tile_pool(name="misc", bufs=6) as mpool,
            tc.tile_pool(name="ps_mm", bufs=2, space="PSUM") as ps_mm,
            tc.tile_pool(name="ps_s", bufs=2, space="PSUM") as ps_s,
            tc.tile_pool(name="ps_o", bufs=2, space="PSUM") as ps_o,
        ):
            # ---- constants / inputs to SBUF ----
            xt_sb = cpool.tile([P, KC, N], F16, name="xt_sb")  # x^T, 8 chunks of [128, 2048]
            wqk_sb = cpool.tile([P, KC, 1024], F16, name="wqk_sb")
            wv_sb = cpool.tile([P, KC, VW], F16, name="wv_sb")
            bqk_sb = cpool.tile([P, 8], F32, name="bqk_sb")
            bv_sb = cpool.tile([P, VW], F16, name="bv_sb")  # b_v (+ones col) replicated
            tri_sb = cpool.tile([P, P], F16, name="tri_sb")
            qt_sb = cpool.tile([P, 4, N], F16, name="qt_sb")  # Q^T pair-stacked
            kt_sb = cpool.tile([P, HL, N], F16, name="kt_sb")
            v_sb = cpool.tile([P, NT, VW], F16, name="v_sb")

            wu_sb = cpool.tile([P, 640], F16, name="wu_sb")  # PE warm-up scratch

            # PE warm-up: the HAM clock gate un-throttles (1.2 -> 2.4 GHz) only after
            # ~3.4us of sustained PE activity; burn that window on scratch matmuls
            # while the input DMAs are in flight.
            nc.gpsimd.memset(wu_sb[:], 0.0)
            psw = ps_mm.tile([P, 512], F32, tag="mm", name="ps_warm")
            NWARM = 12
            for i in range(NWARM):
                nc.tensor.matmul(
                    psw[:],
                    lhsT=wu_sb[:, 0:P],
                    rhs=wu_sb[:, P : P + 512],
                    start=(i == 0),
                    stop=(i == NWARM - 1),
                )

            warm = mpool.tile([1, 8], F32, tag="warm", name="warm")
            nc.gpsimd.memset(warm[:], 0.0)
            nc.scalar.activation(warm[:], warm[:], mybir.ActivationFunctionType.Exp)
            # kt zero-padding memset, split across DVE + gpsimd in slab order so the
            # first S^T (pair 0 -> slabs 0,1) unblocks ~10.5us in instead of ~23:
            # only the actually-zero half-slab of each slab is cleared.
            def zrows(h_l):
                return slice(64, 128) if h_l % 2 == 0 else slice(0, 64)

            for h_l in (0, 1):
                nc.vector.memset(kt_sb[zrows(h_l), h_l, :], 0.0)
            for h_l in range(2, HL):
                nc.gpsimd.memset(kt_sb[zrows(h_l), h_l, :], 0.0)

            x_r = x_d.rearrange("(k p) n -> p k n", p=P)
            # Batched input DMA on the sync ring, ordered by first-use deadline; the
            # start is DMA-bandwidth-bound (8 cores pull at once), so fewer/bigger
            # triggers amortize the ~620ns trigger serialization. (The scalar HWDGE
            # ring measures ~15GB/s vs the sync ring's ~106GB/s — splitting input
            # across both rings starves the start and re-throttles the PE clock.)
            nc.sync.dma_start(bqk_sb[:], bqk_d)
            nc.sync.dma_start(xt_sb[:, 0:4, 0:512], x_r[:, 0:4, 0:512])
            nc.sync.dma_start(wqk_sb[:, 0:4, 0:256], wqk_r[:, 0:4, 0:256])
            nc.sync.dma_start(xt_sb[:, 4:8, 0:512], x_r[:, 4:8, 0:512])
            nc.sync.dma_start(wqk_sb[:, 4:8, 0:256], wqk_r[:, 4:8, 0:256])
            nc.sync.dma_start(tri_sb[:], tri_d)
            nc.sync.dma_start(wv_sb[:, :, :], wv_r[:, :, :])
            nc.sync.dma_start(wqk_sb[:, :, 256:512], wqk_r[:, :, 256:512])
            nc.sync.dma_start(bv_sb[:], bv_d)
            nc.sync.dma_start(xt_sb[:, :, 512:1024], x_r[:, :, 512:1024])
            nc.sync.dma_start(wqk_sb[:, :, 512:768], wqk_r[:, :, 512:768])
            nc.sync.dma_start(wqk_sb[:, :, 768:1024], wqk_r[:, :, 768:1024])
            nc.sync.dma_start(xt_sb[:, :, 1024:1536], x_r[:, :, 1024:1536])
            nc.sync.dma_start(xt_sb[:, :, 1536:2048], x_r[:, :, 1536:2048])

            done_qk = set()
            done_v = set()

            def emit_qk(c, tt):
                if (c, tt) in done_qk:
                    return
                done_qk.add((c, tt))
                pr = c % 4
                pq = ps_mm.tile([P, 512], F32, tag="mm", name=f"pq_{c}_{tt}")
                col0 = 256 * (c % 4) + (0 if c < 4 else 128)
                for k in range(KC):
                    nc.tensor.matmul(
                        pq[:],
                        lhsT=wqk_sb[:, k, col0 : col0 + P],
                        rhs=xt_sb[:, k, tt * 512 : (tt + 1) * 512],
                        start=(k == 0),
                        stop=(k == KC - 1),
                    )
                def badd(out, in_, b):
                    nc.vector.tensor_scalar_add(out, in_, b)

                if c < 4:
                    badd(
                        qt_sb[:, pr, tt * 512 : (tt + 1) * 512], pq[:], bqk_sb[:, c : c + 1]
                    )
                else:
                    for hh in (0, 1):
                        rows = slice(64 * hh, 64 * hh + 64)
                        badd(
                            kt_sb[rows, 2 * pr + hh, tt * 512 : (tt + 1) * 512],
                            pq[rows, :],
                            bqk_sb[rows, c : c + 1],
                        )

            def emit_v(j, half):
                if (j, half) in done_v:
                    return
                done_v.add((j, half))
                pv = ps_mm.tile([P, VH], F32, tag="mm", name=f"pv_{j}_{half}")
                for k in range(KC):
                    nc.tensor.matmul(
                        pv[:],
                        lhsT=xt_sb[:, k, j * P : (j + 1) * P],
                        rhs=wv_sb[:, k, half * VH : (half + 1) * VH],
                        start=(k == 0),
                        stop=(k == KC - 1),
                    )
                nc.vector.tensor_add(
                    v_sb[:, j, half * VH : (half + 1) * VH],
                    pv[:],
                    bv_sb[:, half * VH : (half + 1) * VH],
                )

            filler = []
            for tt in range(4):
                for pr in range(4):
                    if (pr, tt) != (0, 0):
                        filler += [("qk", pr, tt), ("qk", pr + 4, tt)]
                filler += [("v", j, half) for j in range(4 * tt, 4 * tt + 4) for half in (0, 1)]
            state = {"i": 0}

            def pull(n):
                while n > 0 and state["i"] < len(filler):
                    it = filler[state["i"]]
                    state["i"] += 1
                    if it[0] == "v":
                        if (it[1], it[2]) in done_v:
                            continue
                        emit_v(it[1], it[2])
                    else:
                        if (it[1], it[2]) in done_qk:
                            continue
                        emit_qk(it[1], it[2])
                    n -= 1

            def emit_pv_half(p, t, pt, r, hh, ctx, split_dma=False):
                i = 4 * t + r
                if hh == 0:
                    ctx["opair"] = oppool.tile([P, P], F32, tag="op", name=f"op_{p}_{i}")
                    ctx["po"] = po = ps_o.tile([P, 2, 65], F32, tag="o", name=f"po_{p}_{i}")
                else:
                    po = ctx["po"]
                for j in range(i + 1):
                    nc.tensor.matmul(
                        po[:, hh, :],
                        lhsT=pt[:, hh, j, r * P : (r + 1) * P],
                        rhs=v_sb[:, j, 65 * (2 * p + hh) : 65 * (2 * p + hh) + 65],
                        start=(j == 0),
                        stop=(j == i),
                    )
                if hh == 0:
                    return
                opair = ctx["opair"]
                rc = mpool.tile([P, 2], F32, tag="rc", name=f"rc_{p}_{i}")
                nc.vector.reciprocal(rc[:], po[:, :, 64])
                for h2 in (0, 1):
                    if state.get("pos", 0) == 0:
                        nc.scalar.mul(
                            opair[:, 64 * h2 : 64 * h2 + 64],
                            po[:, h2, 0:64],
                            rc[:, h2 : h2 + 1],
                        )
                    else:
                        nc.vector.tensor_scalar_mul(
                            opair[:, 64 * h2 : 64 * h2 + 64], po[:, h2, 0:64], rc[:, h2 : h2 + 1]
                        )
                if split_dma:
                    # final blocks: trigger on the scalar HWDGE ring (idle after the
                    # last exp), avoiding the sync ring's DMA-sem-slot serialization
                    nc.scalar.dma_start(
                        out_d[i * P : (i + 1) * P, p * P : (p + 1) * P], opair[:]
                    )
                else:
                    nc.sync.dma_start(
                        out_d[i * P : (i + 1) * P, p * P : (p + 1) * P], opair[:]
                    )

            def emit_pv(p, t, pt, r, split_dma=False):
                ctx = {}
                emit_pv_half(p, t, pt, r, 0, ctx, split_dma)
                emit_pv_half(p, t, pt, r, 1, ctx, split_dma)

            # Prologue: the first block's QK tiles, emitted as k 0-3 / k 4-7 halves
            # interleaved across both tiles so each half tracks its DMA batch
            # (x stripe-0 and wqk pair-0 arrive as two 4-chunk batches).
            done_qk.update({(0, 0), (4, 0)})
            pq_pro = {
                0: ps_mm.tile([P, 512], F32, tag="mm", name="pq_pro_0"),
                4: ps_mm.tile([P, 512], F32, tag="mm", name="pq_pro_4"),
            }
            for part in (0, 1):
                for c in (0, 4):
                    col0 = 0 if c < 4 else 128
                    for k in range(4 * part, 4 * part + 4):
                        nc.tensor.matmul(
                            pq_pro[c][:],
                            lhsT=wqk_sb[:, k, col0 : col0 + P],
                            rhs=xt_sb[:, k, 0:512],
                            start=(k == 0),
                            stop=(k == KC - 1),
                        )
            nc.vector.tensor_scalar_add(qt_sb[:, 0, 0:512], pq_pro[0][:], bqk_sb[:, 0:1])
            for hh in (0, 1):
                rows = slice(64 * hh, 64 * hh + 64)
                nc.vector.tensor_scalar_add(
                    kt_sb[rows, hh, 0:512], pq_pro[4][rows, :], bqk_sb[rows, 4:5]
                )

            pv_queue = []
            blocks = [(pos, t, p) for pos, t in enumerate((0, 1, 2, 3)) for p in range(4)]
            for n, (pos, t, p) in enumerate(blocks):
                    state["pos"] = pos
                    last = n == len(blocks) - 1
                    for tt in range(t + 1):
                        emit_qk(p, tt)
                        emit_qk(4 + p, tt)
                    nxt_qk = []
                    if n + 1 < len(blocks):
                        _, tn, pn = blocks[n + 1]
                        nxt_qk = [
                            (c, tt)
                            for tt in range(tn + 1)
                            for c in (pn, 4 + pn)
                            if (c, tt) not in done_qk
                        ]
                    pt = ptpool.tile([P, 2, 16, 512], F16, tag="pt", name=f"pt_{p}_{t}")
                    vpend = [
                        (j, half)
                        for j in range(4 * t, 4 * t + 4)
                        for half in (0, 1)
                        if (j, half) not in done_v
                    ]
                    if pos == 3:
                        # ration stripe-3 v chains across the round: emit this
                        # block's needed half fully but at most one chain of the
                        # other half, so later t=3 blocks (whose proj filler is
                        # exhausted) keep PE slack work.
                        mine = [it for it in vpend if it[1] == p // 2]
                        rest = [it for it in vpend if it[1] != p // 2]
                        vpend = mine + rest[: (2 if p == 1 else 0)]

                    def group_hooks(n=n, vpend=vpend, nxt_qk=nxt_qk):
                        if n >= 12:
                            # round 3: pace slack work at one item per group so
                            # it lasts the whole block (the greedy 3-per-group
                            # drain burns it by group ~3 and the PE then stalls
                            # on the exp->psum recycle).
                            if pv_queue:
                                emit_pv(*pv_queue.pop(0))
                            elif vpend:
                                emit_v(*vpend.pop(0))
                            elif nxt_qk:
                                emit_qk(*nxt_qk.pop(0))
                            else:
                                pull(1)
                            return
                        if pv_queue:
                            emit_pv(*pv_queue.pop(0))
                        if vpend:
                            emit_v(*vpend.pop(0))
                        if nxt_qk:
                            emit_qk(*nxt_qk.pop(0))
                            return
                        state["g"] = state.get("g", 0) + 1

                    for g in range(2 * t + 2):
                        psA = ps_s.tile([P, 2, 512], F32, tag="s", name=f"psA_{p}_{t}_{g}")
                        psB = ps_s.tile([P, 2, 512], F32, tag="s", name=f"psB_{p}_{t}_{g}")
                        for jj in (0, 1):
                            j = 2 * g + jj
                            q0 = 128 * (j - 4 * t) if j >= 4 * t else 0
                            for hh, ps in ((0, psA), (1, psB)):
                                nc.tensor.matmul(
                                    ps[:, jj, q0:512],
                                    lhsT=kt_sb[:, 2 * p + hh, j * P : (j + 1) * P],
                                    rhs=qt_sb[:, p, t * 512 + q0 : (t + 1) * 512],
                                    start=True,
                                    stop=True,
                                )
                        for hh, ps in ((0, psA), (1, psB)):
                            if g == 2 * t + 1:
                                # one act over both chunks' 256:512 suffix (chunk
                                # 2g+1's 256:384 is stale finite psum, exp'd but
                                # never consumed; cheaper than a second act)
                                nc.scalar.activation(
                                    pt[:, hh, 2 * g : 2 * g + 2, 256:512],
                                    ps[:, :, 256:512],
                                    mybir.ActivationFunctionType.Exp,
                                    scale=0.125,
                                )
                            elif pos == 3 and g < 2 and hh == 1:
                                # round-3 blocks stay slightly ACT-bound even
                                # with rationed filler; push one head's exp of
                                # the first two (below-diagonal, unmasked)
                                # groups to DVE as single-pass fp16 Schraudolph
                                # fast-exp: int16(184.66*s+15300.6) bitcast fp16
                                # ~ exp(s/8) (~1.8% rms on 1/8 of keys for 1/4
                                # of rows; measured 1.5e-3 end-to-end).
                                nc.vector.tensor_scalar(
                                    pt[:, hh, 2 * g : 2 * g + 2, :].bitcast(
                                        mybir.dt.int16
                                    ),
                                    ps[:], 184.6646003677, 15300.62,
                                    mybir.AluOpType.mult, mybir.AluOpType.add,
                                )
                            else:
                                nc.scalar.activation(
                                    pt[:, hh, 2 * g : 2 * g + 2, :],
                                    ps[:],
                                    mybir.ActivationFunctionType.Exp,
                                    scale=0.125,
                                )
                        group_hooks()
                        if last and g >= 2 * t:
                            for r in (0, 1) if g == 2 * t else (2, 3):
                                j = 4 * t + r
                                for hh in (0, 1):
                                    blk = pt[:, hh, j, r * P : (r + 1) * P]
                                    nc.vector.tensor_mul(blk, blk, tri_sb[:])
                                emit_pv(p, t, pt, r, split_dma=(g == 2 * t + 1))
                    while pv_queue:
                        emit_pv(*pv_queue.pop(0))
                    if last:
                        continue
                    for hh in (0, 1):
                        for r in range(4):
                            j = 4 * t + r
                            blk = pt[:, hh, j, r * P : (r + 1) * P]
                            nc.vector.tensor_mul(blk, blk, tri_sb[:])
                    for j in range(4 * t + 4):
                        emit_v(j, p // 2)
                    pv_queue = [(p, t, pt, r) for r in range(4)]
            while pv_queue:
                emit_pv(*pv_queue.pop(0))
            pull(len(filler))  # safety: flush

    nc.compile()
    return nc


def get_nc():
    if "nc" not in _cache:
        _cache["nc"] = _build()
    return _cache["nc"]


def _prep_core_inputs(x, W, b, bi, hg):
    h0 = hg * HL
    Wq = W[:, 0:D].reshape(D, H, HD)
    Wk = W[:, D : 2 * D].reshape(D, H, HD)
    Wv = W[:, 2 * D :].reshape(D, H, HD)
    bq = b[0:D].reshape(H, HD)
    bk = b[D : 2 * D].reshape(H, HD)
    bv = b[2 * D :].reshape(H, HD)

    wqk = np.empty((D, 1024), np.float32)
    bqk = np.empty((P, 8), np.float32)
    for c in range(4):
        for half in range(2):
            h = h0 + 2 * c + half
            sl = slice(256 * c + half * HD, 256 * c + half * HD + HD)
            wqk[:, sl] = Wq[:, h]
            bqk[half * HD : (half + 1) * HD, c] = bq[h]
            sl = slice(256 * c + P + half * HD, 256 * c + P + half * HD + HD)
            wqk[:, sl] = Wk[:, h]
            bqk[half * HD : (half + 1) * HD, 4 + c] = bk[h]

    wv_aug = np.zeros((D, VW), np.float32)
    bv_aug = np.zeros((VW,), np.float32)
    for hl in range(HL):
        wv_aug[:, 65 * hl : 65 * hl + HD] = Wv[:, h0 + hl]
        bv_aug[65 * hl : 65 * hl + HD] = bv[h0 + hl]
        bv_aug[65 * hl + HD] = 1.0

    tri = np.triu(np.ones((P, P), np.float32))

    return {
        "x": np.ascontiguousarray(x[bi].astype(np.float16).T),
        "wqk": wqk.astype(np.float16),
        "wv": wv_aug.astype(np.float16),
        "bqk": bqk,
        "bv": np.broadcast_to(bv_aug.astype(np.float16), (P, VW)).copy(),
        "tri": tri.astype(np.float16),
    }


def make_in_maps(x, W_qkv, b_qkv):
    x = np.asarray(x, dtype=np.float32)
    W = np.asarray(W_qkv, dtype=np.float32)
    b = np.asarray(b_qkv, dtype=np.float32)
    return [_prep_core_inputs(x, W, b, i // 2, i % 2) for i in range(N_CORES)]


def assemble(results):
    out = np.empty((B, N, D), np.float32)
    for i in range(N_CORES):
        bi, hg = i // 2, i % 2
        out[bi, :, hg * 512 : (hg + 1) * 512] = results[i]["out"]
    return out


def run(x, W_qkv, b_qkv, trace=False, tmpdir=None):
    nc = get_nc()
    in_maps = make_in_maps(x, W_qkv, b_qkv)
    res = bass_utils.run_bass_kernel_spmd(
        nc, in_maps, core_ids=list(range(N_CORES)), trace=trace, tmpdir=tmpdir
    )
    return assemble(res.results), res


def kernel(x, W_qkv, b_qkv):
    out, _ = run(x, W_qkv, b_qkv)
    return out

